# revision 5
# baseline (speedup 1.0000x reference)
"""Bahdanau attention Trainium2 kernel.

Problem: B=32, T=2048, H=F=1024
  q = query @ Wa_w.T + Wa_b            [B,1,H]
  k = keys @ Ua_w.T + Ua_b             [B,T,H]
  e = tanh(q + k)                      [B,T,H]
  scores = e @ Va_w[0] (+ Va_b)        [B,T]   (Va_b drops out of softmax)
  weights = softmax(scores)            [B,1,T]
  context = weights @ keys             [B,1,F]
returns (context, weights)

Sharding: data-parallel over batch, 4 batches per core on 8 cores.
Device math in bf16 with fp32 PSUM accumulation (validated ~2.6e-3
scale-relative absmax vs the fp32 reference).

Per-core device pipeline (per batch):
  kk^T[o,t] accumulated in PSUM via lhsT=UaT chunks, rhs=keysT chunks
  ACT tanh(psum + q[o]) with q as per-partition bias -> e^T bf16
  scores[1,t] via lhsT=Va[128,1] (M=1), rhs=e^T     (PE)
  ACT Exp with accum_out -> unnormalized w row + partition-0 sums
  w row -> (DRAM bounce) -> w columns [128,16] bf16 (t = p*16+c)
  context[1,f] via lhsT=w column, rhs=keys chunks; scaled by 1/Z
"""

import numpy as np
import ml_dtypes

import concourse.bass as bass
import concourse.mybir as mybir
import concourse.tile as tile
from concourse import bacc
from concourse.bass import ts, ds
from concourse.bass_utils import run_bass_kernel_spmd

B, T, H, F = 32, 2048, 1024, 1024
NCORES = 8
BPC = B // NCORES  # batches per core
P = 128
KF = F // P   # 8 contraction chunks over feature dim
KO = H // P   # 8 chunks over hidden (o) dim
TC = T // P   # 16 t-chunks of 128 (for t = p*16 + c layout, c in 0..15)

F32 = mybir.dt.float32
BF16 = mybir.dt.bfloat16
AF = mybir.ActivationFunctionType

LAST_RESULTS = None  # BassKernelResults from the most recent run (for test.py)


def build_nc():
    nc = bacc.Bacc()

    # ---- external I/O (per-core shapes; layouts premapped on host so every
    # DMA is contiguous per partition) ----
    keysT_d = nc.dram_tensor("keysT", [BPC, P, KF, T], BF16, kind="ExternalInput")
    keysN_d = nc.dram_tensor("keysN", [BPC, P, TC, F], BF16, kind="ExternalInput")
    uaT_d = nc.dram_tensor("uaT", [P, KF, H], BF16, kind="ExternalInput")
    waT_d = nc.dram_tensor("waT", [P, KO, H], BF16, kind="ExternalInput")
    qT_d = nc.dram_tensor("qT", [P, KO, BPC], BF16, kind="ExternalInput")
    va_d = nc.dram_tensor("va", [P, KO], BF16, kind="ExternalInput")
    biasq_d = nc.dram_tensor("biasq", [P, KO], F32, kind="ExternalInput")
    ctx_d = nc.dram_tensor("ctx", [BPC, 1, F], F32, kind="ExternalOutput")
    wts_d = nc.dram_tensor("wts", [BPC, 1, T], F32, kind="ExternalOutput")

    with tile.TileContext(nc) as tc:
        with (
            tc.tile_pool(name="const", bufs=1) as const,
            tc.tile_pool(name="ktp", bufs=2) as ktp,
            tc.tile_pool(name="kp", bufs=1) as kp,
            tc.tile_pool(name="etp", bufs=1) as etp,
            tc.tile_pool(name="rows", bufs=1) as rows,
            tc.tile_pool(name="tiny", bufs=2) as tiny,
            tc.tile_pool(name="dram", bufs=2, space="DRAM") as dramp,
            tc.tile_pool(name="mpsum", bufs=3, space="PSUM") as mpsum,
            tc.tile_pool(name="spsum", bufs=2, space="PSUM") as spsum,
        ):
            # ---- constants ----
            ua_sb = const.tile([P, KF, H], BF16)
            nc.sync.dma_start(ua_sb, uaT_d[:])
            wa_sb = const.tile([P, KO, H], BF16)
            nc.sync.dma_start(wa_sb, waT_d[:])
            qT_sb = const.tile([P, KO, BPC], BF16)
            nc.sync.dma_start(qT_sb, qT_d[:])
            va_sb = const.tile([P, KO], BF16)
            nc.sync.dma_start(va_sb, va_d[:])
            biasq_sb = const.tile([P, KO], F32)
            nc.sync.dma_start(biasq_sb, biasq_d[:])

            # ---- q^T[o, b] = sum_h Wa[o,h] query[b,h]  -> [128, KO, BPC] f32
            qb_sb = const.tile([P, KO, BPC], F32)
            for m in range(KO):
                qps = spsum.tile([P, BPC], F32, tag="sm")
                for k in range(KO):
                    nc.tensor.matmul(
                        qps,
                        lhsT=wa_sb[:, k, ts(m, P)],
                        rhs=qT_sb[:, k, :],
                        start=(k == 0),
                        stop=(k == KO - 1),
                    )
                # psum -> sbuf with the (Wa_b + Ua_b) bias folded in
                nc.scalar.add(qb_sb[:, m, :], qps, biasq_sb[:, m : m + 1])

            for b in range(BPC):
                ktT = ktp.tile([P, KF, T], BF16)  # keys^T (f on partitions)
                nc.sync.dma_start(ktT, keysT_d[b])
                ks = kp.tile([P, TC, F], BF16)  # keys natural (t = p*16+c)
                nc.sync.dma_start(ks, keysN_d[b])
                eT = etp.tile([P, KO, T], BF16)  # e^T (o on partitions)

                # ---- main matmul + fused q-bias tanh ----
                # kk^T[o, t] = sum_f UaT[f,o] * keysT[f,t]
                for m in range(KO):
                    for th in range(2):  # t halves of 1024
                        ps = mpsum.tile([P, 1024], F32, tag="mp")
                        for k in range(KF):
                            for n2 in range(2):
                                nc.tensor.matmul(
                                    ps[:, ts(n2, 512)],
                                    lhsT=ua_sb[:, k, ts(m, P)],
                                    rhs=ktT[:, k, ds(th * 1024 + n2 * 512, 512)],
                                    start=(k == 0),
                                    stop=(k == KF - 1),
                                )
                        nc.scalar.activation(
                            eT[:, m, ds(th * 1024, 1024)],
                            ps[:],
                            AF.Tanh,
                            bias=qb_sb[:, m, b : b + 1],
                        )

                # ---- scores + exp (unnormalized softmax) ----
                wexp = rows.tile([1, T], F32)
                zparts = tiny.tile([1, 4], F32)
                for ns in range(4):
                    sps = spsum.tile([1, 512], F32, tag="sm")
                    for k in range(KO):
                        nc.tensor.matmul(
                            sps,
                            lhsT=va_sb[:, k : k + 1],
                            rhs=eT[:, k, ts(ns, 512)],
                            start=(k == 0),
                            stop=(k == KO - 1),
                        )
                    nc.scalar.activation(
                        wexp[:, ts(ns, 512)],
                        sps[:],
                        AF.Exp,
                        accum_out=zparts[:, ns : ns + 1],
                    )
                z = tiny.tile([1, 1], F32)
                nc.vector.reduce_sum(z, zparts[:], axis=mybir.AxisListType.X)
                rz = tiny.tile([1, 1], F32)
                nc.vector.reciprocal(rz, z[:])

                # normalized weights row -> output
                wrow = rows.tile([1, T], F32)
                nc.scalar.mul(wrow, wexp[:], rz[:])
                nc.sync.dma_start(wts_d[b], wrow)

                # w columns for the context matmul: wcol[p, c] = wexp[p*16+c]
                # (DRAM bounce; SWDGE casts f32 -> bf16 on the way back)
                wtmp = dramp.tile([1, T], F32)
                nc.sync.dma_start(wtmp, wexp[:])
                wcol = tiny.tile([P, TC], BF16)
                nc.gpsimd.dma_start(
                    wcol, wtmp.rearrange("a (p c) -> (a p) c", p=P)
                )

                # ---- context[1, f] = (1/Z) * sum_t wexp[t] keys[t, f] ----
                crow = rows.tile([1, F], F32)
                for n2 in range(2):
                    cps = spsum.tile([1, 512], F32, tag="sm")
                    for c in range(TC):
                        nc.tensor.matmul(
                            cps,
                            lhsT=wcol[:, c : c + 1],
                            rhs=ks[:, c, ts(n2, 512)],
                            start=(c == 0),
                            stop=(c == TC - 1),
                        )
                    nc.vector.tensor_scalar_mul(crow[:, ts(n2, 512)], cps[:], rz[:])
                nc.sync.dma_start(ctx_d[b], crow)

    if not nc.is_finalized():
        nc.finalize()
    return nc


def _prep_inputs(query, keys, Wa_w, Wa_b, Ua_w, Ua_b, Va_w):
    """Host-side layout prep. Returns per-core input maps."""
    bf = ml_dtypes.bfloat16
    query = np.asarray(query, np.float32)
    keys = np.asarray(keys, np.float32)

    kb = keys.astype(bf)  # [B, T, F]
    # keysT[b, p, k, t] = keys[b, t, k*128+p]
    keysT = np.ascontiguousarray(
        kb.transpose(0, 2, 1).reshape(B, KF, P, T).transpose(0, 2, 1, 3)
    )
    # keysN[b, p, c, f] = keys[b, p*16+c, f]
    keysN = kb.reshape(B, P, TC, F)  # contiguous already

    # uaT[p, k, o] = Ua[o, k*128+p]
    uaT = np.ascontiguousarray(
        np.asarray(Ua_w, np.float32).astype(bf).T.reshape(KF, P, H).transpose(1, 0, 2)
    )
    waT = np.ascontiguousarray(
        np.asarray(Wa_w, np.float32).astype(bf).T.reshape(KO, P, H).transpose(1, 0, 2)
    )
    va = np.ascontiguousarray(
        np.asarray(Va_w, np.float32)[0].astype(bf).reshape(KO, P).T
    )
    biasq = np.ascontiguousarray(
        (np.asarray(Wa_b, np.float32) + np.asarray(Ua_b, np.float32))
        .reshape(KO, P)
        .T
    )

    q_all = query[:, 0, :].astype(bf)  # [B, H]
    in_maps = []
    for c in range(NCORES):
        sl = slice(c * BPC, (c + 1) * BPC)
        # qT[p, k, b] = query[b, k*128+p]
        qT = np.ascontiguousarray(q_all[sl].reshape(BPC, KO, P).transpose(2, 1, 0))
        in_maps.append(
            {
                "keysT": keysT[sl],
                "keysN": keysN[sl],
                "uaT": uaT,
                "waT": waT,
                "qT": qT,
                "va": va,
                "biasq": biasq,
            }
        )
    return in_maps


def kernel(query, keys, Wa_w, Wa_b, Ua_w, Ua_b, Va_w, Va_b, **_ignored):
    global LAST_RESULTS
    in_maps = _prep_inputs(query, keys, Wa_w, Wa_b, Ua_w, Ua_b, Va_w)
    nc = build_nc()
    res = run_bass_kernel_spmd(nc, in_maps, list(range(NCORES)))
    LAST_RESULTS = res
    ctx = np.concatenate([np.asarray(r["ctx"], np.float32) for r in res.results], 0)
    wts = np.concatenate([np.asarray(r["wts"], np.float32) for r in res.results], 0)
    return ctx, wts


# revision 8
# speedup vs baseline: 190.6172x; 190.6172x over previous
"""Bahdanau attention Trainium2 kernel.

Problem: B=32, T=2048, H=F=1024
  q = query @ Wa_w.T + Wa_b            [B,1,H]
  k = keys @ Ua_w.T + Ua_b             [B,T,H]
  e = tanh(q + k)                      [B,T,H]
  scores = e @ Va_w[0] (+ Va_b)        [B,T]   (Va_b drops out of softmax)
  weights = softmax(scores)            [B,1,T]
  context = weights @ keys             [B,1,F]
returns (context, weights)

Sharding: data-parallel over batch, 4 batches per core on 8 cores.
Device math in bf16 with fp32 PSUM accumulation (validated ~2.6e-3
scale-relative absmax vs the fp32 reference).

Per-core device pipeline (per batch):
  kk^T[o,t] accumulated in PSUM via lhsT=UaT chunks, rhs=keysT chunks
  ACT tanh(psum + q[o]) with q as per-partition bias -> e^T bf16
  scores[1,t] via lhsT=Va[128,1] (M=1), rhs=e^T     (PE)
  ACT Exp with accum_out -> unnormalized w row + partition-0 sums
  w row -> (DRAM bounce) -> w columns [128,16] bf16 (t = p*16+c)
  context[1,f] via lhsT=w column, rhs=keys chunks; scaled by 1/Z
"""

import numpy as np
import ml_dtypes

import concourse.bass as bass
import concourse.mybir as mybir
import concourse.tile as tile
from concourse import bacc
from concourse.bass import ts, ds
from concourse.bass_utils import run_bass_kernel_spmd

B, T, H, F = 32, 2048, 1024, 1024
NCORES = 8
BPC = B // NCORES  # batches per core
P = 128
KF = F // P   # 8 contraction chunks over feature dim
KO = H // P   # 8 chunks over hidden (o) dim
TC = T // P   # 16 t-chunks of 128 (for t = p*16 + c layout, c in 0..15)

F32 = mybir.dt.float32
BF16 = mybir.dt.bfloat16
AF = mybir.ActivationFunctionType

LAST_RESULTS = None  # BassKernelResults from the most recent run (for test.py)


def build_nc(repeat=1):
    """repeat>1 wraps the per-batch pipeline in a device-side loop; used only
    for timing (slope of wall time vs repeat isolates HW kernel time from the
    axon RPC overhead)."""
    nc = bacc.Bacc()

    # ---- external I/O (per-core shapes; layouts premapped on host so every
    # DMA is contiguous per partition) ----
    keysT_d = nc.dram_tensor("keysT", [BPC, P, KF, T], BF16, kind="ExternalInput")
    keysN_d = nc.dram_tensor("keysN", [BPC, P, TC, F], BF16, kind="ExternalInput")
    uaT_d = nc.dram_tensor("uaT", [P, KF, H], BF16, kind="ExternalInput")
    waT_d = nc.dram_tensor("waT", [P, KO, H], BF16, kind="ExternalInput")
    qT_d = nc.dram_tensor("qT", [P, KO, BPC], BF16, kind="ExternalInput")
    va_d = nc.dram_tensor("va", [P, KO], BF16, kind="ExternalInput")
    biasq_d = nc.dram_tensor("biasq", [P, KO], F32, kind="ExternalInput")
    ctx_d = nc.dram_tensor("ctx", [BPC, 1, F], F32, kind="ExternalOutput")
    wts_d = nc.dram_tensor("wts", [BPC, 1, T], F32, kind="ExternalOutput")

    with tile.TileContext(nc) as tc:
        with (
            tc.tile_pool(name="const", bufs=1) as const,
            tc.tile_pool(name="ktp", bufs=2) as ktp,
            tc.tile_pool(name="kp", bufs=1) as kp,
            tc.tile_pool(name="etp", bufs=1) as etp,
            tc.tile_pool(name="rows", bufs=1) as rows,
            tc.tile_pool(name="tiny", bufs=2) as tiny,
            tc.tile_pool(name="dram", bufs=2, space="DRAM") as dramp,
            tc.tile_pool(name="mpsum", bufs=3, space="PSUM") as mpsum,
            tc.tile_pool(name="spsum", bufs=2, space="PSUM") as spsum,
        ):
            # ---- constants ----
            ua_sb = const.tile([P, KF, H], BF16)
            nc.sync.dma_start(ua_sb, uaT_d[:])
            wa_sb = const.tile([P, KO, H], BF16)
            nc.sync.dma_start(wa_sb, waT_d[:])
            qT_sb = const.tile([P, KO, BPC], BF16)
            nc.sync.dma_start(qT_sb, qT_d[:])
            va_sb = const.tile([P, KO], BF16)
            nc.sync.dma_start(va_sb, va_d[:])
            biasq_sb = const.tile([P, KO], F32)
            nc.sync.dma_start(biasq_sb, biasq_d[:])

            # ---- q^T[o, b] = sum_h Wa[o,h] query[b,h]  -> [128, KO, BPC] f32
            qb_sb = const.tile([P, KO, BPC], F32)
            for m in range(KO):
                qps = spsum.tile([P, BPC], F32, tag="sm")
                for k in range(KO):
                    nc.tensor.matmul(
                        qps,
                        lhsT=wa_sb[:, k, ts(m, P)],
                        rhs=qT_sb[:, k, :],
                        start=(k == 0),
                        stop=(k == KO - 1),
                    )
                # psum -> sbuf with the (Wa_b + Ua_b) bias folded in
                nc.scalar.add(qb_sb[:, m, :], qps, biasq_sb[:, m : m + 1])

            def batch_pipeline():
              for b in range(BPC):
                ktT = ktp.tile([P, KF, T], BF16)  # keys^T (f on partitions)
                nc.sync.dma_start(ktT, keysT_d[b])
                ks = kp.tile([P, TC, F], BF16)  # keys natural (t = p*16+c)
                nc.sync.dma_start(ks, keysN_d[b])
                eT = etp.tile([P, KO, T], BF16)  # e^T (o on partitions)

                # ---- main matmul + fused q-bias tanh ----
                # kk^T[o, t] = sum_f UaT[f,o] * keysT[f,t]
                for m in range(KO):
                    for th in range(2):  # t halves of 1024
                        ps = mpsum.tile([P, 1024], F32, tag="mp")
                        for k in range(KF):
                            for n2 in range(2):
                                nc.tensor.matmul(
                                    ps[:, ts(n2, 512)],
                                    lhsT=ua_sb[:, k, ts(m, P)],
                                    rhs=ktT[:, k, ds(th * 1024 + n2 * 512, 512)],
                                    start=(k == 0),
                                    stop=(k == KF - 1),
                                )
                        nc.scalar.activation(
                            eT[:, m, ds(th * 1024, 1024)],
                            ps[:],
                            AF.Tanh,
                            bias=qb_sb[:, m, b : b + 1],
                        )

                # ---- scores + exp (unnormalized softmax) ----
                wexp = rows.tile([1, T], F32)
                zparts = tiny.tile([1, 4], F32)
                for ns in range(4):
                    sps = spsum.tile([1, 512], F32, tag="sm")
                    for k in range(KO):
                        nc.tensor.matmul(
                            sps,
                            lhsT=va_sb[:, k : k + 1],
                            rhs=eT[:, k, ts(ns, 512)],
                            start=(k == 0),
                            stop=(k == KO - 1),
                        )
                    nc.scalar.activation(
                        wexp[:, ts(ns, 512)],
                        sps[:],
                        AF.Exp,
                        accum_out=zparts[:, ns : ns + 1],
                    )
                z = tiny.tile([1, 1], F32)
                nc.vector.reduce_sum(z, zparts[:], axis=mybir.AxisListType.X)
                rz = tiny.tile([1, 1], F32)
                nc.vector.reciprocal(rz, z[:])

                # normalized weights row -> output
                wrow = rows.tile([1, T], F32)
                nc.scalar.mul(wrow, wexp[:], rz[:])
                nc.sync.dma_start(wts_d[b], wrow)

                # w columns for the context matmul: wcol[p, c] = wexp[p*16+c]
                # (DRAM bounce; SWDGE casts f32 -> bf16 on the way back)
                wtmp = dramp.tile([1, T], F32)
                nc.sync.dma_start(wtmp, wexp[:])
                wcol = tiny.tile([P, TC], BF16)
                nc.gpsimd.dma_start(
                    wcol, wtmp.rearrange("a (p c) -> (a p) c", p=P)
                )

                # ---- context[1, f] = (1/Z) * sum_t wexp[t] keys[t, f] ----
                crow = rows.tile([1, F], F32)
                for n2 in range(2):
                    cps = spsum.tile([1, 512], F32, tag="sm")
                    for c in range(TC):
                        nc.tensor.matmul(
                            cps,
                            lhsT=wcol[:, c : c + 1],
                            rhs=ks[:, c, ts(n2, 512)],
                            start=(c == 0),
                            stop=(c == TC - 1),
                        )
                    nc.vector.tensor_scalar_mul(crow[:, ts(n2, 512)], cps[:], rz[:])
                nc.sync.dma_start(ctx_d[b], crow)

            if repeat == 1:
                batch_pipeline()
            else:
                with tc.For_i(0, repeat, 1):
                    batch_pipeline()

    if not nc.is_finalized():
        nc.finalize()
    return nc


def _prep_inputs(query, keys, Wa_w, Wa_b, Ua_w, Ua_b, Va_w):
    """Host-side layout prep. Returns per-core input maps."""
    bf = ml_dtypes.bfloat16
    query = np.asarray(query, np.float32)
    keys = np.asarray(keys, np.float32)

    kb = keys.astype(bf)  # [B, T, F]
    # keysT[b, p, k, t] = keys[b, t, k*128+p]
    keysT = np.ascontiguousarray(
        kb.transpose(0, 2, 1).reshape(B, KF, P, T).transpose(0, 2, 1, 3)
    )
    # keysN[b, p, c, f] = keys[b, p*16+c, f]
    keysN = kb.reshape(B, P, TC, F)  # contiguous already

    # uaT[p, k, o] = Ua[o, k*128+p]
    uaT = np.ascontiguousarray(
        np.asarray(Ua_w, np.float32).astype(bf).T.reshape(KF, P, H).transpose(1, 0, 2)
    )
    waT = np.ascontiguousarray(
        np.asarray(Wa_w, np.float32).astype(bf).T.reshape(KO, P, H).transpose(1, 0, 2)
    )
    va = np.ascontiguousarray(
        np.asarray(Va_w, np.float32)[0].astype(bf).reshape(KO, P).T
    )
    biasq = np.ascontiguousarray(
        (np.asarray(Wa_b, np.float32) + np.asarray(Ua_b, np.float32))
        .reshape(KO, P)
        .T
    )

    q_all = query[:, 0, :].astype(bf)  # [B, H]
    in_maps = []
    for c in range(NCORES):
        sl = slice(c * BPC, (c + 1) * BPC)
        # qT[p, k, b] = query[b, k*128+p]
        qT = np.ascontiguousarray(q_all[sl].reshape(BPC, KO, P).transpose(2, 1, 0))
        in_maps.append(
            {
                "keysT": keysT[sl],
                "keysN": keysN[sl],
                "uaT": uaT,
                "waT": waT,
                "qT": qT,
                "va": va,
                "biasq": biasq,
            }
        )
    return in_maps


def kernel(query, keys, Wa_w, Wa_b, Ua_w, Ua_b, Va_w, Va_b, **_ignored):
    global LAST_RESULTS
    in_maps = _prep_inputs(query, keys, Wa_w, Wa_b, Ua_w, Ua_b, Va_w)
    nc = build_nc()
    res = run_bass_kernel_spmd(nc, in_maps, list(range(NCORES)))
    LAST_RESULTS = res
    ctx = np.concatenate([np.asarray(r["ctx"], np.float32) for r in res.results], 0)
    wts = np.concatenate([np.asarray(r["wts"], np.float32) for r in res.results], 0)
    return ctx, wts


# revision 11
# speedup vs baseline: 272.3616x; 1.4288x over previous
"""Bahdanau attention Trainium2 kernel.

Problem: B=32, T=2048, H=F=1024
  q = query @ Wa_w.T + Wa_b            [B,1,H]
  k = keys @ Ua_w.T + Ua_b             [B,T,H]
  e = tanh(q + k)                      [B,T,H]
  scores = e @ Va_w[0] (+ Va_b)        [B,T]   (Va_b drops out of softmax)
  weights = softmax(scores)            [B,1,T]
  context = weights @ keys             [B,1,F]
returns (context, weights)

Sharding: data-parallel over batch, 4 batches per core on 8 cores.
Device math in bf16 with fp32 PSUM accumulation (validated ~2.6e-3
scale-relative absmax vs the fp32 reference).

Per-core device pipeline (per batch):
  kk^T[o,t] accumulated in PSUM via lhsT=UaT chunks, rhs=keysT chunks
  ACT tanh(psum + q[o]) with q as per-partition bias -> e^T bf16
  scores[1,t] via lhsT=Va[128,1] (M=1), rhs=e^T     (PE)
  ACT Exp with accum_out -> unnormalized w row + partition-0 sums
  w row -> (DRAM bounce) -> w columns [128,16] bf16 (t = p*16+c)
  context[1,f] via lhsT=w column, rhs=keys chunks; scaled by 1/Z
"""

import numpy as np
import ml_dtypes

import concourse.bass as bass
import concourse.mybir as mybir
import concourse.tile as tile
from concourse import bacc
from concourse.bass import ts, ds
from concourse.bass_utils import run_bass_kernel_spmd

B, T, H, F = 32, 2048, 1024, 1024
NCORES = 8
BPC = B // NCORES  # batches per core
P = 128
KF = F // P   # 8 contraction chunks over feature dim
KO = H // P   # 8 chunks over hidden (o) dim
TC = T // P   # 16 t-chunks of 128 (for t = p*16 + c layout, c in 0..15)

F32 = mybir.dt.float32
BF16 = mybir.dt.bfloat16
AF = mybir.ActivationFunctionType

LAST_RESULTS = None  # BassKernelResults from the most recent run (for test.py)


def build_nc(repeat=1):
    """repeat>1 wraps the per-batch pipeline in a device-side loop; used only
    for timing (slope of wall time vs repeat isolates HW kernel time from the
    axon RPC overhead)."""
    nc = bacc.Bacc()

    # ---- external I/O (per-core shapes; layouts premapped on host so every
    # DMA is contiguous per partition) ----
    keysT_d = nc.dram_tensor("keysT", [BPC, P, KF, T], BF16, kind="ExternalInput")
    keysN_d = nc.dram_tensor("keysN", [BPC, P, TC, F], BF16, kind="ExternalInput")
    uaT_d = nc.dram_tensor("uaT", [P, KF, H], BF16, kind="ExternalInput")
    waT_d = nc.dram_tensor("waT", [P, KO, H], BF16, kind="ExternalInput")
    qT_d = nc.dram_tensor("qT", [P, KO, BPC], BF16, kind="ExternalInput")
    va_d = nc.dram_tensor("va", [P, KO], BF16, kind="ExternalInput")
    biasq_d = nc.dram_tensor("biasq", [P, KO], F32, kind="ExternalInput")
    ctx_d = nc.dram_tensor("ctx", [BPC, 1, F], F32, kind="ExternalOutput")
    wts_d = nc.dram_tensor("wts", [BPC, 1, T], F32, kind="ExternalOutput")

    with tile.TileContext(nc) as tc:
        with (
            tc.tile_pool(name="const", bufs=1) as const,
            tc.tile_pool(name="ktp", bufs=2) as ktp,
            tc.tile_pool(name="kp", bufs=2) as kp,
            tc.tile_pool(name="etp", bufs=1) as etp,
            tc.tile_pool(name="rows", bufs=1) as rows,
            tc.tile_pool(name="tiny", bufs=2) as tiny,
            tc.tile_pool(name="dram", bufs=2, space="DRAM") as dramp,
            tc.tile_pool(name="mpsum", bufs=3, space="PSUM") as mpsum,
            tc.tile_pool(name="spsum", bufs=2, space="PSUM") as spsum,
        ):
            # ---- constants ----
            # wa is only needed for the q phase; borrow a keysT-pool slot so
            # its SBUF space is reclaimed for keysT double-buffering.
            wa_sb = ktp.tile([P, KO, H], BF16, tag="ktT")
            nc.sync.dma_start(wa_sb, waT_d[:])
            qT_sb = const.tile([P, KO, BPC], BF16)
            nc.sync.dma_start(qT_sb, qT_d[:])
            va_sb = const.tile([P, KO], BF16)
            nc.sync.dma_start(va_sb, va_d[:])
            biasq_sb = const.tile([P, KO], F32)
            nc.sync.dma_start(biasq_sb, biasq_d[:])
            ua_sb = const.tile([P, KF, H], BF16)
            nc.sync.dma_start(ua_sb, uaT_d[:])

            # ---- q^T[o, b] = sum_h Wa[o,h] query[b,h]  -> [128, KO, BPC] f32
            qb_sb = const.tile([P, KO, BPC], F32)
            for m in range(KO):
                qps = spsum.tile([P, BPC], F32, tag="sm")
                for k in range(KO):
                    nc.tensor.matmul(
                        qps,
                        lhsT=wa_sb[:, k, ts(m, P)],
                        rhs=qT_sb[:, k, :],
                        start=(k == 0),
                        stop=(k == KO - 1),
                    )
                # psum -> sbuf with the (Wa_b + Ua_b) bias folded in
                nc.scalar.add(qb_sb[:, m, :], qps, biasq_sb[:, m : m + 1])

            def emit_context(pend):
                """context[1, f] = (1/Z) * sum_t wexp[t] keys[t, f].
                Emitted after the NEXT batch's main matmuls so the PE never
                stalls on the softmax/wcol chain."""
                b, ks, wcol, rz = pend
                crow = rows.tile([1, F], F32)
                for n2 in range(2):
                    cps = spsum.tile([1, 512], F32, tag="sm")
                    for c in range(TC):
                        nc.tensor.matmul(
                            cps,
                            lhsT=wcol[:, c : c + 1],
                            rhs=ks[:, c, ts(n2, 512)],
                            start=(c == 0),
                            stop=(c == TC - 1),
                        )
                    nc.vector.tensor_scalar_mul(crow[:, ts(n2, 512)], cps[:], rz[:])
                nc.sync.dma_start(ctx_d[b], crow)

            def batch_pipeline():
              pending = None
              for b in range(BPC):
                ktT = ktp.tile([P, KF, T], BF16, tag="ktT")  # keys^T (f on part.)
                nc.sync.dma_start(ktT, keysT_d[b])
                ks = kp.tile([P, TC, F], BF16)  # keys natural (t = p*16+c)
                nc.sync.dma_start(ks, keysN_d[b])
                eT = etp.tile([P, KO, T], BF16)  # e^T (o on partitions)

                # ---- main matmul + fused q-bias tanh ----
                # kk^T[o, t] = sum_f UaT[f,o] * keysT[f,t]
                for m in range(KO):
                    for th in range(2):  # t halves of 1024
                        ps = mpsum.tile([P, 1024], F32, tag="mp")
                        for k in range(KF):
                            for n2 in range(2):
                                nc.tensor.matmul(
                                    ps[:, ts(n2, 512)],
                                    lhsT=ua_sb[:, k, ts(m, P)],
                                    rhs=ktT[:, k, ds(th * 1024 + n2 * 512, 512)],
                                    start=(k == 0),
                                    stop=(k == KF - 1),
                                )
                        nc.scalar.activation(
                            eT[:, m, ds(th * 1024, 1024)],
                            ps[:],
                            AF.Tanh,
                            bias=qb_sb[:, m, b : b + 1],
                        )

                # previous batch's context (its wcol/rz resolved long ago)
                if pending is not None:
                    emit_context(pending)

                # ---- scores + exp (unnormalized softmax) ----
                wexp = rows.tile([1, T], F32)
                zparts = tiny.tile([1, 4], F32)
                for ns in range(4):
                    sps = spsum.tile([1, 512], F32, tag="sm")
                    for k in range(KO):
                        nc.tensor.matmul(
                            sps,
                            lhsT=va_sb[:, k : k + 1],
                            rhs=eT[:, k, ts(ns, 512)],
                            start=(k == 0),
                            stop=(k == KO - 1),
                        )
                    nc.scalar.activation(
                        wexp[:, ts(ns, 512)],
                        sps[:],
                        AF.Exp,
                        accum_out=zparts[:, ns : ns + 1],
                    )
                z = tiny.tile([1, 1], F32)
                nc.vector.reduce_sum(z, zparts[:], axis=mybir.AxisListType.X)
                rz = tiny.tile([1, 1], F32)
                nc.vector.reciprocal(rz, z[:])

                # w columns for the context matmul: wcol[p, c] = wexp[p*16+c]
                # (DRAM bounce; SWDGE casts f32 -> bf16 on the way back)
                wtmp = dramp.tile([1, T], F32)
                nc.sync.dma_start(wtmp, wexp[:])
                wcol = tiny.tile([P, TC], BF16)
                nc.gpsimd.dma_start(
                    wcol, wtmp.rearrange("a (p c) -> (a p) c", p=P)
                )

                # normalized weights row -> output (in-place; after the bounce
                # has read the unnormalized values)
                nc.scalar.mul(wexp[:], wexp[:], rz[:])
                nc.sync.dma_start(wts_d[b], wexp)

                pending = (b, ks, wcol, rz)
              emit_context(pending)

            if repeat == 1:
                batch_pipeline()
            else:
                with tc.For_i(0, repeat, 1):
                    batch_pipeline()

    if not nc.is_finalized():
        nc.finalize()
    return nc


def _prep_inputs(query, keys, Wa_w, Wa_b, Ua_w, Ua_b, Va_w):
    """Host-side layout prep. Returns per-core input maps."""
    bf = ml_dtypes.bfloat16
    query = np.asarray(query, np.float32)
    keys = np.asarray(keys, np.float32)

    kb = keys.astype(bf)  # [B, T, F]
    # keysT[b, p, k, t] = keys[b, t, k*128+p]
    keysT = np.ascontiguousarray(
        kb.transpose(0, 2, 1).reshape(B, KF, P, T).transpose(0, 2, 1, 3)
    )
    # keysN[b, p, c, f] = keys[b, p*16+c, f]
    keysN = kb.reshape(B, P, TC, F)  # contiguous already

    # uaT[p, k, o] = Ua[o, k*128+p]
    uaT = np.ascontiguousarray(
        np.asarray(Ua_w, np.float32).astype(bf).T.reshape(KF, P, H).transpose(1, 0, 2)
    )
    waT = np.ascontiguousarray(
        np.asarray(Wa_w, np.float32).astype(bf).T.reshape(KO, P, H).transpose(1, 0, 2)
    )
    va = np.ascontiguousarray(
        np.asarray(Va_w, np.float32)[0].astype(bf).reshape(KO, P).T
    )
    biasq = np.ascontiguousarray(
        (np.asarray(Wa_b, np.float32) + np.asarray(Ua_b, np.float32))
        .reshape(KO, P)
        .T
    )

    q_all = query[:, 0, :].astype(bf)  # [B, H]
    in_maps = []
    for c in range(NCORES):
        sl = slice(c * BPC, (c + 1) * BPC)
        # qT[p, k, b] = query[b, k*128+p]
        qT = np.ascontiguousarray(q_all[sl].reshape(BPC, KO, P).transpose(2, 1, 0))
        in_maps.append(
            {
                "keysT": keysT[sl],
                "keysN": keysN[sl],
                "uaT": uaT,
                "waT": waT,
                "qT": qT,
                "va": va,
                "biasq": biasq,
            }
        )
    return in_maps


def kernel(query, keys, Wa_w, Wa_b, Ua_w, Ua_b, Va_w, Va_b, **_ignored):
    global LAST_RESULTS
    in_maps = _prep_inputs(query, keys, Wa_w, Wa_b, Ua_w, Ua_b, Va_w)
    nc = build_nc()
    res = run_bass_kernel_spmd(nc, in_maps, list(range(NCORES)))
    LAST_RESULTS = res
    ctx = np.concatenate([np.asarray(r["ctx"], np.float32) for r in res.results], 0)
    wts = np.concatenate([np.asarray(r["wts"], np.float32) for r in res.results], 0)
    return ctx, wts
